# revision 8
# baseline (speedup 1.0000x reference)
"""HMM forward-algorithm (MsaHmmCell) Trainium2 kernel.

Full inputs in, full outputs out. Internally: data-parallel over batch across
8 NeuronCores (32 sequences per core); each core holds the full transition /
emission matrices in SBUF and runs the T=512-step sequential scan.

Device-side formulation (per core):
  - state kept TRANSPOSED: s_t stored (128 part = q mod 128, 9*32 free) where
    free index j = 32*c + b encodes q-chunk c (q = 128*c + p) and batch b.
  - transition: s@A as 81 accumulating matmuls, out = lhsT.T @ rhs with
    lhsT = A-tile (k-chunk part, m-chunk free) resident bf16 (FWL), rhs =
    s-chunk (128, 32). Output stays transposed -> no per-step transposes.
  - emission E_t = B.T[x_t] via one-hot matmul (BT-chunk lhsT, onehot rhs).
  - normalization is NOT done per step on device. Instead every step is
    rescaled by r = 1/sigma from TWO steps earlier (folded into the emission
    scale via a K=1 broadcast matmul), which keeps magnitudes bounded while
    keeping the reciprocal off the PE critical path. Stored per-step column
    sums sigma and the applied r let the host reconstruct the exactly
    normalized forward variables and the log-likelihood.
"""

import os

import numpy as np
import ml_dtypes

import concourse.bass as bass
import concourse.mybir as mybir
import concourse.tile as tile
from concourse.bass_utils import run_bass_kernel_spmd

Q = 1027
ALPHA = 26
BATCH = 256
T = 512
NCORES = 8
BC = BATCH // NCORES          # 32 sequences per core
NCH = 9                       # q chunks
QP = NCH * 128                # padded state count 1152
SW = NCH * BC                 # state free width 288

BF16 = mybir.dt.bfloat16
F32 = mybir.dt.float32

LAST_RESULTS = None           # BassKernelResults of the most recent run (for test.py)
TRACE = bool(os.environ.get("HMM_TRACE"))


def _split_drain_waits(nc, max_waits=1):
    """This walrus build encodes at most one sync wait per instruction.
    Move excess waits onto preceding single-wait NoOps (same engine, in-order
    execution => semantically identical)."""
    for f in nc.m.functions:
        for bb in f.blocks:
            new_insts = []
            for inst in bb.instructions:
                si = getattr(inst, "sync_info", None)
                if (
                    si is not None
                    and si.on_wait
                    and len(si.on_wait) > max_waits
                ):
                    waits = list(si.on_wait)
                    excess, keep = waits[:-max_waits], waits[-max_waits:]
                    for w in excess:
                        new_insts.append(
                            mybir.InstNoOp(
                                name=f"I-{nc.next_id()}",
                                engine=inst.engine,
                                ins=[],
                                outs=[],
                                sync_info=mybir.SyncInfo(on_wait=[w], on_update=[]),
                            )
                        )
                    inst.sync_info = mybir.SyncInfo(
                        on_wait=keep, on_update=list(si.on_update or [])
                    )
                new_insts.append(inst)
            bb.instructions[:] = new_insts


def _build(nsteps=T):
    nc = bass.Bass("TRN2", target_bir_lowering=False, debug=False)

    a_in = nc.dram_tensor("a_t", (128, NCH * NCH * 128), BF16, kind="ExternalInput")
    bt_in = nc.dram_tensor("bt_t", (ALPHA, QP), BF16, kind="ExternalInput")
    oh_in = nc.dram_tensor("oh_t", (ALPHA, nsteps * BC), BF16, kind="ExternalInput")
    init_in = nc.dram_tensor("init_t", (128, NCH), F32, kind="ExternalInput")

    s_out = nc.dram_tensor("s_out", (nsteps, 128, SW), BF16, kind="ExternalOutput")
    m_out = nc.dram_tensor("m_out", (nsteps, BC), F32, kind="ExternalOutput")
    r_out = nc.dram_tensor("r_out", (nsteps, BC), BF16, kind="ExternalOutput")

    with tile.TileContext(nc) as tc:
        with (
            tc.tile_pool(name="const", bufs=1) as const,
            tc.tile_pool(name="state", bufs=3) as state,
            tc.tile_pool(name="emis", bufs=2) as emis,
            tc.tile_pool(name="small", bufs=3) as small,
            tc.tile_pool(name="ph", bufs=2, space="PSUM") as ph_pool,
            tc.tile_pool(name="pe", bufs=2, space="PSUM") as pe_pool,
            tc.tile_pool(name="pbc", bufs=2, space="PSUM") as pbc_pool,
            tc.tile_pool(name="psig", bufs=2, space="PSUM") as psig_pool,
        ):
            # ---- resident constants ----
            a_sb = const.tile([128, NCH * NCH * 128], BF16)
            nc.gpsimd.dma_start(out=a_sb, in_=a_in[:, :])
            bt_sb = const.tile([ALPHA, QP], BF16)
            nc.gpsimd.dma_start(out=bt_sb, in_=bt_in[:, :])
            oh_sb = const.tile([ALPHA, nsteps * BC], BF16)
            nc.gpsimd.dma_start(out=oh_sb, in_=oh_in[:, :])
            init_sb = const.tile([128, NCH], F32)
            nc.gpsimd.dma_start(out=init_sb, in_=init_in[:, :])
            ones_k = const.tile([128, 1], BF16)
            nc.vector.memset(ones_k, 1.0)
            ones_m = const.tile([1, 128], BF16)
            nc.vector.memset(ones_m, 1.0)

            def emm(t):
                """emission matmuls for step t -> psum tile, plus copy to sbuf"""
                pE = pe_pool.tile([128, SW], F32, tag="pE")
                for m in range(NCH):
                    nc.tensor.matmul(
                        pE[:, m * BC : (m + 1) * BC],
                        bt_sb[:, m * 128 : (m + 1) * 128],
                        oh_sb[:, t * BC : (t + 1) * BC],
                        start=True,
                        stop=True,
                    )
                E = emis.tile([128, SW], BF16, tag="E")
                nc.scalar.copy(out=E, in_=pE)
                return E

            def sigma(t, s):
                """column sums of s -> psum (1, BC); write m_out[t]"""
                psig = psig_pool.tile([1, BC], F32, tag="psig")
                for c in range(NCH):
                    nc.tensor.matmul(
                        psig,
                        ones_k,
                        s[:, c * BC : (c + 1) * BC],
                        start=(c == 0),
                        stop=(c == NCH - 1),
                    )
                msb = small.tile([1, BC], F32, tag="msb")
                nc.scalar.copy(out=msb, in_=psig)
                nc.gpsimd.dma_start(out=m_out[t : t + 1, :], in_=msb)
                return psig

            def recip(t, psig):
                r = small.tile([1, BC], F32, tag="rsb")
                nc.vector.reciprocal(out=r, in_=psig)
                # bf16 copy: the value actually applied on-device AND stored for
                # the host bookkeeping, so the rounding cancels exactly.
                rb = small.tile([1, BC], BF16, tag="rbf")
                nc.vector.tensor_copy(out=rb, in_=r)
                nc.gpsimd.dma_start(out=r_out[t : t + 1, :], in_=rb)
                return rb

            def bcast(r):
                pbc = pbc_pool.tile([128, SW], F32, tag="pbc")
                for c in range(NCH):
                    nc.tensor.matmul(
                        pbc[:, c * BC : (c + 1) * BC], ones_m, r, start=True, stop=True
                    )
                return pbc

            # ---- t = 0 ----
            E0 = emm(0)
            s_prev = state.tile([128, SW], BF16, tag="s")
            for c in range(NCH):
                nc.vector.tensor_scalar_mul(
                    out=s_prev[:, c * BC : (c + 1) * BC],
                    in0=E0[:, c * BC : (c + 1) * BC],
                    scalar1=init_sb[:, c : c + 1],
                )
            nc.gpsimd.dma_start(out=s_out[0], in_=s_prev)
            psig = sigma(0, s_prev)
            r0 = recip(0, psig)
            pbc = bcast(r0)
            E1 = emm(1)
            esc_prev = emis.tile([128, SW], BF16, tag="esc")
            nc.vector.tensor_mul(out=esc_prev, in0=E1, in1=pbc)
            r_prev = r0  # r_{t-1} available at entry of iteration t

            # ---- t = 1 .. nsteps-1 ----
            for t in range(1, nsteps):
                # transition: ps_h[:, m] += A[k,m].T @ s_prev[:, k]
                # k-outer order: the MMs consuming state chunk k are emitted
                # together, so the chunked state-multiply below lets next-step
                # MMs start as soon as chunk 0 is written.
                ph = ph_pool.tile([128, SW], F32, tag="ph")
                for k in range(NCH):
                    for m in range(NCH):
                        nc.tensor.matmul(
                            ph[:, m * BC : (m + 1) * BC],
                            a_sb[:, (k * NCH + m) * 128 : (k * NCH + m + 1) * 128],
                            s_prev[:, k * BC : (k + 1) * BC],
                            # start clears has_written for the WHOLE bank:
                            # only the very first MM of the step may set it;
                            # later MMs overwrite-where-clear / add-where-set.
                            start=(k == 0 and m == 0),
                            stop=(k == NCH - 1 and m == NCH - 1),
                            skip_group_check=True,
                        )
                # s_t = ph * Esc_t   (DVE; one PSUM operand), in 3 chunks to
                # release consumers early
                s_cur = state.tile([128, SW], BF16, tag="s")
                for j in range(3):
                    sl = slice(j * (SW // 3), (j + 1) * (SW // 3))
                    nc.vector.tensor_mul(
                        out=s_cur[:, sl], in0=ph[:, sl], in1=esc_prev[:, sl]
                    )
                nc.gpsimd.dma_start(out=s_out[t], in_=s_cur)

                if t < nsteps - 1:
                    # independent PE work to cover the DVE latency
                    E_next = emm(t + 1)
                    pbc = bcast(r_prev)  # r_{t-1} -> scales E_{t+1}
                psig = sigma(t, s_cur)
                if t < nsteps - 1:
                    r_prev = recip(t, psig)
                    esc_prev = emis.tile([128, SW], BF16, tag="esc")
                    nc.vector.tensor_mul(out=esc_prev, in0=E_next, in1=pbc)
                s_prev = s_cur

    _split_drain_waits(nc)
    return nc


def _softmax(x, axis=-1):
    x = np.asarray(x, dtype=np.float32)
    m = x.max(axis=axis, keepdims=True)
    e = np.exp(x - m, dtype=np.float32)
    return e / e.sum(axis=axis, keepdims=True, dtype=np.float32)


_CACHE = {}


def kernel(x, A_logits, B_logits, init_logits):
    global LAST_RESULTS
    x = np.asarray(x, dtype=np.int32)
    A_logits = np.asarray(A_logits, dtype=np.float32)
    B_logits = np.asarray(B_logits, dtype=np.float32)
    init_logits = np.asarray(init_logits, dtype=np.float32)

    # ---- host prep ----
    A = _softmax(A_logits)                       # (Q, Q)
    Bm = _softmax(B_logits)                      # (Q, ALPHA)
    init = _softmax(init_logits)                 # (Q,)

    A_pad = np.zeros((QP, QP), np.float32)
    A_pad[:Q, :Q] = A
    # a_host[p, k, m, c] = A_pad[k*128+p, m*128+c]
    a_host = np.ascontiguousarray(
        A_pad.reshape(NCH, 128, NCH, 128).transpose(1, 0, 2, 3)
    ).reshape(128, NCH * NCH * 128).astype(ml_dtypes.bfloat16)

    bt_host = np.zeros((ALPHA, QP), np.float32)
    bt_host[:, :Q] = Bm.T
    bt_host = bt_host.astype(ml_dtypes.bfloat16)

    init_pad = np.zeros((QP,), np.float32)
    init_pad[:Q] = init
    init_host = np.ascontiguousarray(init_pad.reshape(NCH, 128).T)  # (128, NCH)

    in_maps = []
    for i in range(NCORES):
        xi = x[i * BC : (i + 1) * BC]            # (BC, T)
        oh = (
            np.arange(ALPHA, dtype=np.int32).reshape(ALPHA, 1, 1)
            == xi.T.reshape(1, T, BC)
        )
        in_maps.append(
            {
                "a_t": a_host,
                "bt_t": bt_host,
                "oh_t": np.ascontiguousarray(oh.reshape(ALPHA, T * BC)).astype(
                    ml_dtypes.bfloat16
                ),
                "init_t": init_host,
            }
        )

    if "nc" not in _CACHE:
        _CACHE["nc"] = _build(T)
    nc = _CACHE["nc"]

    res = run_bass_kernel_spmd(
        nc, in_maps, core_ids=list(range(NCORES)), trace=TRACE
    )
    LAST_RESULTS = res

    # ---- host reconstruction ----
    forward = np.empty((BATCH, T, Q), np.float32)
    loglik = np.empty((BATCH, 1), np.float32)
    # index of the r applied at step t (folded into E_t): r_{max(t-2, 0)}
    ridx = np.maximum(np.arange(1, T) - 2, 0)
    for i in range(NCORES):
        out = res.results[i]
        s = out["s_out"].astype(np.float32)      # (T, 128, SW)
        mm = out["m_out"].astype(np.float64)     # (T, BC)
        rr = out["r_out"].astype(np.float64)     # (T, BC)
        # forward[b, t, q=c*128+p] = s[t, p, c*32+b] / m[t, b]
        sr = s.reshape(T, 128, NCH, BC).transpose(3, 0, 2, 1).reshape(BC, T, QP)
        fw = sr[:, :, :Q] / mm.T[:, :, None].astype(np.float32)
        forward[i * BC : (i + 1) * BC] = fw
        # log S_t: S_0 = m_0; S_t = m_t / (r_{ridx(t)} * m_{t-1})
        logm = np.log(mm)                        # (T, BC)
        logr = np.log(np.where(rr > 0, rr, 1.0))  # (T, BC); last rows unused
        ll = logm[0] + np.sum(logm[1:] - logm[:-1] - logr[ridx], axis=0)
        loglik[i * BC : (i + 1) * BC, 0] = ll.astype(np.float32)

    return forward, loglik


# revision 9
# speedup vs baseline: 1.1288x; 1.1288x over previous
"""HMM forward-algorithm (MsaHmmCell) Trainium2 kernel.

Full inputs in, full outputs out. Internally: data-parallel over batch across
8 NeuronCores (32 sequences per core); each core holds the full transition /
emission matrices in SBUF and runs the T=512-step sequential scan.

Device-side formulation (per core):
  - state kept TRANSPOSED: s_t stored (128 part = q mod 128, 9*32 free) where
    free index j = 32*c + b encodes q-chunk c (q = 128*c + p) and batch b.
  - transition: s@A as 81 accumulating matmuls, out = lhsT.T @ rhs with
    lhsT = A-tile (k-chunk part, m-chunk free) resident bf16 (FWL), rhs =
    s-chunk (128, 32). Output stays transposed -> no per-step transposes.
  - emission E_t = B.T[x_t] via one-hot matmul (BT-chunk lhsT, onehot rhs).
  - normalization is NOT done per step on device. Instead every step is
    rescaled by r = 1/sigma from TWO steps earlier (folded into the emission
    scale via a K=1 broadcast matmul), which keeps magnitudes bounded while
    keeping the reciprocal off the PE critical path. Stored per-step column
    sums sigma and the applied r let the host reconstruct the exactly
    normalized forward variables and the log-likelihood.
"""

import os

import numpy as np
import ml_dtypes

import concourse.bass as bass
import concourse.mybir as mybir
import concourse.tile as tile
from concourse.bass_utils import run_bass_kernel_spmd

Q = 1027
ALPHA = 26
BATCH = 256
T = 512
NCORES = 8
BC = BATCH // NCORES          # 32 sequences per core
NCH = 9                       # q chunks
QP = NCH * 128                # padded state count 1152
SW = NCH * BC                 # state free width 288

BF16 = mybir.dt.bfloat16
F32 = mybir.dt.float32

LAST_RESULTS = None           # BassKernelResults of the most recent run (for test.py)
TRACE = bool(os.environ.get("HMM_TRACE"))


def _split_drain_waits(nc, max_waits=1):
    """This walrus build encodes at most one sync wait per instruction.
    Move excess waits onto preceding single-wait NoOps (same engine, in-order
    execution => semantically identical)."""
    for f in nc.m.functions:
        for bb in f.blocks:
            new_insts = []
            for inst in bb.instructions:
                si = getattr(inst, "sync_info", None)
                if (
                    si is not None
                    and si.on_wait
                    and len(si.on_wait) > max_waits
                ):
                    waits = list(si.on_wait)
                    excess, keep = waits[:-max_waits], waits[-max_waits:]
                    for w in excess:
                        new_insts.append(
                            mybir.InstNoOp(
                                name=f"I-{nc.next_id()}",
                                engine=inst.engine,
                                ins=[],
                                outs=[],
                                sync_info=mybir.SyncInfo(on_wait=[w], on_update=[]),
                            )
                        )
                    inst.sync_info = mybir.SyncInfo(
                        on_wait=keep, on_update=list(si.on_update or [])
                    )
                new_insts.append(inst)
            bb.instructions[:] = new_insts


def _build(nsteps=T):
    nc = bass.Bass("TRN2", target_bir_lowering=False, debug=False)

    a_in = nc.dram_tensor("a_t", (128, NCH * NCH * 128), BF16, kind="ExternalInput")
    e_in = nc.dram_tensor("e_t", (nsteps, 128, SW), BF16, kind="ExternalInput")
    init_in = nc.dram_tensor("init_t", (128, NCH), F32, kind="ExternalInput")

    s_out = nc.dram_tensor("s_out", (nsteps, 128, SW), BF16, kind="ExternalOutput")
    m_out = nc.dram_tensor("m_out", (nsteps, BC), F32, kind="ExternalOutput")
    r_out = nc.dram_tensor("r_out", (nsteps, BC), BF16, kind="ExternalOutput")

    CH0 = 2 * BC  # first state-multiply chunk (released early for next step)

    with tile.TileContext(nc) as tc:
        with (
            tc.tile_pool(name="const", bufs=1) as const,
            tc.tile_pool(name="state", bufs=3) as state,
            tc.tile_pool(name="emis", bufs=3) as emis,
            tc.tile_pool(name="small", bufs=3) as small,
            tc.tile_pool(name="ph", bufs=2, space="PSUM") as ph_pool,
            tc.tile_pool(name="pbc", bufs=2, space="PSUM") as pbc_pool,
            tc.tile_pool(name="psig", bufs=2, space="PSUM") as psig_pool,
        ):
            # ---- resident constants ----
            a_sb = const.tile([128, NCH * NCH * 128], BF16)
            nc.gpsimd.dma_start(out=a_sb, in_=a_in[:, :])
            init_sb = const.tile([128, NCH], F32)
            nc.gpsimd.dma_start(out=init_sb, in_=init_in[:, :])
            ones_k = const.tile([128, 1], BF16)
            nc.vector.memset(ones_k, 1.0)
            ones_m = const.tile([1, 128], BF16)
            nc.vector.memset(ones_m, 1.0)

            def eload(t):
                E = emis.tile([128, SW], BF16, tag="E")
                nc.sync.dma_start(out=E, in_=e_in[t])
                return E

            def sigma(t, s):
                """column sums of s -> psum (1, BC); write m_out[t]"""
                psig = psig_pool.tile([1, BC], F32, tag="psig")
                for c in range(NCH):
                    nc.tensor.matmul(
                        psig,
                        ones_k,
                        s[:, c * BC : (c + 1) * BC],
                        start=(c == 0),
                        stop=(c == NCH - 1),
                    )
                msb = small.tile([1, BC], F32, tag="msb")
                nc.scalar.copy(out=msb, in_=psig)
                nc.sync.dma_start(out=m_out[t : t + 1, :], in_=msb)
                return psig

            def recip(t, psig):
                # bf16 reciprocal: the value applied on-device IS the value the
                # host uses, so its rounding cancels in the reconstruction.
                rb = small.tile([1, BC], BF16, tag="rbf")
                with nc.allow_low_precision("r rounding tracked exactly on host"):
                    nc.vector.reciprocal(out=rb, in_=psig)
                nc.sync.dma_start(out=r_out[t : t + 1, :], in_=rb)
                return rb

            def bcast(r):
                """broadcast r to all partitions/chunks as bf16 in SBUF"""
                pbc = pbc_pool.tile([128, SW], F32, tag="pbc")
                for c in range(NCH):
                    nc.tensor.matmul(
                        pbc[:, c * BC : (c + 1) * BC], ones_m, r, start=True, stop=True
                    )
                rbc = emis.tile([128, SW], BF16, tag="rbc")
                nc.scalar.copy(out=rbc, in_=pbc)
                return rbc

            # ---- t = 0 ----
            E0 = eload(0)
            s_prev = state.tile([128, SW], BF16, tag="s")
            for c in range(NCH):
                nc.vector.tensor_scalar_mul(
                    out=s_prev[:, c * BC : (c + 1) * BC],
                    in0=E0[:, c * BC : (c + 1) * BC],
                    scalar1=init_sb[:, c : c + 1],
                )
            nc.sync.dma_start(out=s_out[0], in_=s_prev)
            psig = sigma(0, s_prev)
            r_prev = recip(0, psig)       # r_{t-1} at entry of iteration t
            rbc = bcast(r_prev)
            E1 = eload(1)
            esc_prev = emis.tile([128, SW], BF16, tag="esc")
            nc.vector.tensor_mul(out=esc_prev, in0=E1, in1=rbc)

            # ---- t = 1 .. nsteps-1 ----
            for t in range(1, nsteps):
                # transition: ps_h[:, m] += A[k,m].T @ s_prev[:, k]
                # k-outer: MMs consuming state chunk k are adjacent, so the
                # chunked multiply below releases next-step MMs early.
                ph = ph_pool.tile([128, SW], F32, tag="ph")
                for k in range(NCH):
                    for m in range(NCH):
                        nc.tensor.matmul(
                            ph[:, m * BC : (m + 1) * BC],
                            a_sb[:, (k * NCH + m) * 128 : (k * NCH + m + 1) * 128],
                            s_prev[:, k * BC : (k + 1) * BC],
                            # start clears has_written for the WHOLE bank: only
                            # the first MM of the step may set it.
                            start=(k == 0 and m == 0),
                            stop=(k == NCH - 1 and m == NCH - 1),
                            skip_group_check=True,
                        )
                # s_t = ph * Esc_t (DVE, PSUM operand => 1x mode); small first
                # chunk so next-step k=0/1 MMs start early
                s_cur = state.tile([128, SW], BF16, tag="s")
                nc.vector.tensor_mul(
                    out=s_cur[:, :CH0], in0=ph[:, :CH0], in1=esc_prev[:, :CH0]
                )
                nc.vector.tensor_mul(
                    out=s_cur[:, CH0:], in0=ph[:, CH0:], in1=esc_prev[:, CH0:]
                )
                nc.sync.dma_start(out=s_out[t], in_=s_cur)

                if t < nsteps - 1:
                    E_next = eload(t + 1)
                    rbc = bcast(r_prev)   # r_{t-1} -> scales E_{t+1}
                psig = sigma(t, s_cur)
                if t < nsteps - 1:
                    r_prev = recip(t, psig)
                    esc_prev = emis.tile([128, SW], BF16, tag="esc")
                    nc.vector.tensor_mul(out=esc_prev, in0=E_next, in1=rbc)
                s_prev = s_cur

    _split_drain_waits(nc)
    return nc


def _softmax(x, axis=-1):
    x = np.asarray(x, dtype=np.float32)
    m = x.max(axis=axis, keepdims=True)
    e = np.exp(x - m, dtype=np.float32)
    return e / e.sum(axis=axis, keepdims=True, dtype=np.float32)


_CACHE = {}


def kernel(x, A_logits, B_logits, init_logits):
    global LAST_RESULTS
    x = np.asarray(x, dtype=np.int32)
    A_logits = np.asarray(A_logits, dtype=np.float32)
    B_logits = np.asarray(B_logits, dtype=np.float32)
    init_logits = np.asarray(init_logits, dtype=np.float32)

    # ---- host prep ----
    A = _softmax(A_logits)                       # (Q, Q)
    Bm = _softmax(B_logits)                      # (Q, ALPHA)
    init = _softmax(init_logits)                 # (Q,)

    A_pad = np.zeros((QP, QP), np.float32)
    A_pad[:Q, :Q] = A
    # a_host[p, k, m, c] = A_pad[k*128+p, m*128+c]
    a_host = np.ascontiguousarray(
        A_pad.reshape(NCH, 128, NCH, 128).transpose(1, 0, 2, 3)
    ).reshape(128, NCH * NCH * 128).astype(ml_dtypes.bfloat16)

    Bpad = np.zeros((QP, ALPHA), np.float32)
    Bpad[:Q] = Bm

    init_pad = np.zeros((QP,), np.float32)
    init_pad[:Q] = init
    init_host = np.ascontiguousarray(init_pad.reshape(NCH, 128).T)  # (128, NCH)

    in_maps = []
    for i in range(NCORES):
        xi = x[i * BC : (i + 1) * BC]            # (BC, T)
        # E_host[t, p, c*BC+b] = Bpad[128c+p, x[b,t]]
        e = Bpad[:, xi]                          # (QP, BC, T)
        e = e.reshape(NCH, 128, BC, T).transpose(3, 1, 0, 2).reshape(T, 128, SW)
        in_maps.append(
            {
                "a_t": a_host,
                "e_t": np.ascontiguousarray(e).astype(ml_dtypes.bfloat16),
                "init_t": init_host,
            }
        )

    if "nc" not in _CACHE:
        _CACHE["nc"] = _build(T)
    nc = _CACHE["nc"]

    res = run_bass_kernel_spmd(
        nc, in_maps, core_ids=list(range(NCORES)), trace=TRACE
    )
    LAST_RESULTS = res

    # ---- host reconstruction ----
    forward = np.empty((BATCH, T, Q), np.float32)
    loglik = np.empty((BATCH, 1), np.float32)
    # index of the r applied at step t (folded into E_t): r_{max(t-2, 0)}
    ridx = np.maximum(np.arange(1, T) - 2, 0)
    for i in range(NCORES):
        out = res.results[i]
        s = out["s_out"].astype(np.float32)      # (T, 128, SW)
        mm = out["m_out"].astype(np.float64)     # (T, BC)
        rr = out["r_out"].astype(np.float64)     # (T, BC)
        # forward[b, t, q=c*128+p] = s[t, p, c*32+b] / m[t, b]
        sr = s.reshape(T, 128, NCH, BC).transpose(3, 0, 2, 1).reshape(BC, T, QP)
        fw = sr[:, :, :Q] / mm.T[:, :, None].astype(np.float32)
        forward[i * BC : (i + 1) * BC] = fw
        # log S_t: S_0 = m_0; S_t = m_t / (r_{ridx(t)} * m_{t-1})
        logm = np.log(mm)                        # (T, BC)
        logr = np.log(np.where(rr > 0, rr, 1.0))  # (T, BC); last rows unused
        ll = logm[0] + np.sum(logm[1:] - logm[:-1] - logr[ridx], axis=0)
        loglik[i * BC : (i + 1) * BC, 0] = ll.astype(np.float32)

    return forward, loglik
